# revision 17
# baseline (speedup 1.0000x reference)
"""Trainium2 Bass kernel: DeepSeekV2 MLA attention block (T=S=2048, H=16).

Sharding: 2 heads per core (16 heads / 8 cores); kv latents replicated;
row-parallel wo (each core computes a full [T, DIM] partial using its
heads' slice of wo); host sums the 8 partials.

Per-core pipeline:
  1. decompress k_nopeT per head from kv latents with fp8 DoubleRow
     matmuls (256-wide contraction, 2x-4x rate); v in fp16 + transposes
  2. transposed-logits attention: ONE fp8 DoubleRow matmul per s-chunk
     computes nope+rope logits fused (k-tile 0 = k_nope x q_nope,
     k-tile 1 = pe x q_pe), exp on ACT (logits are tiny -> no max
     subtraction), causal mask via affine_select on diagonal-crossing
     chunks, denominator via ones-matmul, PV accumulation in fp16
  3. normalize with partition-broadcast reciprocal, row-parallel wo

fp8 is safe for the logit side only: logits*SCALE ~ N(0, 0.002), so
~4% fp8 relative noise on logits perturbs softmax weights by ~1e-4
relative.  v / PV / wo stay fp16 (their fp8 noise would hit the output
at full strength).

t-tiles are [512,512,512,384,128] so the serial tail after the last
attention chunk (wo matmuls + output DMA of the final tile) is 4x
smaller than with uniform 512 tiles.
"""
import sys

for _p in ("/opt/trn_rl_repo", "/root/.axon_site/_ro/trn_rl_repo"):
    if _p not in sys.path:
        sys.path.insert(0, _p)

import ml_dtypes
import numpy as np

import concourse.bass as bass  # noqa: F401  (registers engines)
import concourse.tile as tile
from concourse import bacc, mybir
from concourse.bass_utils import run_bass_kernel_spmd
from concourse.masks import make_identity

T = 2048
S = 2048
H = 16
DN = 128
DR = 64
DV = 128
CLR = 512
DIM = 2048
NCORES = 8
HL = H // NCORES          # heads per core
SCALE = 1.0 / float(np.sqrt(DN + DR))

f32 = mybir.dt.float32
f16 = mybir.dt.float16
f8 = mybir.dt.float8e4
DR_MODE = mybir.MatmulPerfMode.DoubleRow

NC_S = S // 128           # 16 s-chunks of 128
NCC = CLR // 128          # 4 latent chunks of 128
NB = S // 512             # 4 s-blocks of 512 (decompress granularity)
NM = DIM // 512           # 4 output dim tiles of 512
TILES = [(0, 512), (512, 512), (1024, 512), (1536, 384), (1920, 128)]

# fp8 scale factors; nope product (S_QN*S_KV*S_WK) == rope product
# (S_QP*S_PE) so one DoubleRow matmul can sum both k-tiles.
S_KV = 8.0
S_WK = 16.0
S_QN = 32.0
S_QP = 64.0
S_PE = 64.0
EXP_SCALE = SCALE / (S_QN * S_KV * S_WK)

np8 = ml_dtypes.float8_e4m3

_CACHE = {}


def _build(pcl: int):
    nc = bacc.Bacc("TRN2", target_bir_lowering=False, debug=False,
                   num_devices=NCORES)

    kv8_d = nc.dram_tensor("kv8", [128, NB, NCC, 512], f8,
                           kind="ExternalInput").ap()
    kv16_d = nc.dram_tensor("kv16", [128, NB, NCC, 512], f16,
                            kind="ExternalInput").ap()
    wk8_d = nc.dram_tensor("wk8", [128, HL, NCC, DN], f8,
                           kind="ExternalInput").ap()
    wv16_d = nc.dram_tensor("wv16", [128, HL, NCC, DV], f16,
                            kind="ExternalInput").ap()
    qpk8_d = nc.dram_tensor("qpk8", [128, HL, 2, T], f8,
                            kind="ExternalInput").ap()
    pe8_d = nc.dram_tensor("pe8", [128, S], f8, kind="ExternalInput").ap()
    woT_d = nc.dram_tensor("woT", [128, HL, DIM], f16,
                           kind="ExternalInput").ap()
    ones_d = nc.dram_tensor("ones", [128, 128], f16, kind="ExternalInput").ap()
    ones8_d = nc.dram_tensor("ones8", [128, 2, 128], f8,
                             kind="ExternalInput").ap()
    out_d = nc.dram_tensor("out", [T, DIM], f16, kind="ExternalOutput").ap()

    with tile.TileContext(nc) as tc:
        with tc.tile_pool(name="singles", bufs=1) as singles:
            # --- resident SBUF state; DMA priority order matters ---
            wk8_sb = singles.tile([128, HL, NCC, DN], f8)
            nc.sync.dma_start(wk8_sb[:], wk8_d)
            kv8_sb = singles.tile([128, NB, NCC, 512], f8)
            kv16_sb = singles.tile([128, NB, NCC, 512], f16)
            # per-block loads, first blocks first, spread across rings;
            # block 0 split in halves so the first matmuls can start sooner
            nc.gpsimd.dma_start(kv8_sb[:, 0, 0:2], kv8_d[:, 0, 0:2])
            nc.scalar.dma_start(kv8_sb[:, 0, 2:4], kv8_d[:, 0, 2:4])
            nc.sync.dma_start(kv16_sb[:, 0, 0:2], kv16_d[:, 0, 0:2])
            nc.gpsimd.dma_start(kv16_sb[:, 0, 2:4], kv16_d[:, 0, 2:4])
            wv16_sb = singles.tile([128, HL, NCC, DV], f16)
            nc.scalar.dma_start(wv16_sb[:], wv16_d)
            # kn_pack rows: 0 = kn_h0, 1 = pe (DMA), 2 = kn_h1
            kn_pack = singles.tile([128, 3, S], f8)
            nc.sync.dma_start(kn_pack[:, 1, :], pe8_d)
            nc.gpsimd.dma_start(kv8_sb[:, 1], kv8_d[:, 1])
            nc.scalar.dma_start(kv16_sb[:, 1], kv16_d[:, 1])
            ones_sb = singles.tile([128, 128], f16)
            nc.sync.dma_start(ones_sb[:], ones_d)
            ones8_sb = singles.tile([128, 2, 128], f8)
            nc.sync.dma_start(ones8_sb[:], ones8_d)
            nc.gpsimd.dma_start(kv8_sb[:, 2], kv8_d[:, 2])
            nc.scalar.dma_start(kv16_sb[:, 2], kv16_d[:, 2])
            nc.sync.dma_start(kv8_sb[:, 3], kv8_d[:, 3])
            nc.gpsimd.dma_start(kv16_sb[:, 3], kv16_d[:, 3])
            qpk_sb = singles.tile([128, HL, 2, T], f8)
            nc.sync.dma_start(qpk_sb[:], qpk8_d)
            wo_sb = singles.tile([128, HL, DIM], f16)
            nc.scalar.dma_start(wo_sb[:], woT_d)
            ident = singles.tile([128, 128], f16)
            make_identity(nc, ident[:])

            v_sb = [singles.tile([128, S], f16, tag=f"v{h}", name=f"v{h}")
                    for h in range(HL)]
            wo_h = [wo_sb[:, h, :] for h in range(HL)]

            # --- phase 0: decompress k_nopeT (fp8) and v (fp16) ---
            with tc.tile_pool(name="dec_ps", bufs=2, space="PSUM") as dec_ps, \
                 tc.tile_pool(name="tp_ps", bufs=2, space="PSUM") as tp_ps, \
                 tc.tile_pool(name="vstage", bufs=2) as vstage:
                for st in range(NB):
                    sl = slice(st * 512, (st + 1) * 512)
                    for h in range(HL):
                        kp = dec_ps.tile([128, 512], f32, tag="kp")
                        for t2 in range(NCC // 2):
                            nc.tensor.matmul(
                                kp[:], wk8_sb[:, h, 2 * t2:2 * t2 + 2, :],
                                kv8_sb[:, st, 2 * t2:2 * t2 + 2, :],
                                start=(t2 == 0), stop=(t2 == NCC // 2 - 1),
                                perf_mode=DR_MODE)
                        # f32 -> f8 cast straight into the packed lhsT
                        nc.vector.tensor_copy(kn_pack[:, 2 * h, sl], kp[:])
                    for h in range(HL):
                        vp = dec_ps.tile([128, 512], f32, tag="vp")
                        for c in range(NCC):
                            nc.tensor.matmul(vp[:], wv16_sb[:, h, c, :],
                                             kv16_sb[:, st, c, :],
                                             start=(c == 0),
                                             stop=(c == NCC - 1))
                        vs = vstage.tile([128, 512], f16)
                        nc.scalar.copy(vs[:], vp[:])
                        tp = tp_ps.tile([128, 512], f16)
                        for b in range(4):
                            nc.tensor.transpose(
                                tp[:, b * 128:(b + 1) * 128],
                                vs[:, b * 128:(b + 1) * 128], ident[:])
                        if (st + h) % 2 == 0:
                            nc.vector.tensor_copy(v_sb[h][:, sl], tp[:])
                        else:
                            nc.scalar.copy(v_sb[h][:, sl], tp[:])

            # --- phase 1: attention + wo (software-pipelined) ---
            with tc.tile_pool(name="lg_ps", bufs=2, space="PSUM") as lg_ps, \
                 tc.tile_pool(name="dn_ps", bufs=2, space="PSUM") as dn_ps, \
                 tc.tile_pool(name="ov_ps", bufs=2, space="PSUM") as ov_ps, \
                 tc.tile_pool(name="wo_ps", bufs=2, space="PSUM") as wo_ps, \
                 tc.tile_pool(name="pT", bufs=5) as p_pool, \
                 tc.tile_pool(name="pq", bufs=4) as p8_pool, \
                 tc.tile_pool(name="recip", bufs=2) as r_pool, \
                 tc.tile_pool(name="ovn", bufs=6) as ovn_pool, \
                 tc.tile_pool(name="osb", bufs=4) as out_pool:
                ovn_tiles = {}
                out_rings = [nc.sync, nc.gpsimd]
                ring_ctr = [0]

                def emit_wo(j, last=False):
                    t0, tw = TILES[j]
                    for q in range(tw // 128):
                        for m in range(NM):
                            msl = slice(m * 512, (m + 1) * 512)
                            wp = wo_ps.tile([128, 512], f32, name="wp")
                            for h in range(HL):
                                nc.tensor.matmul(
                                    wp[:],
                                    ovn_tiles[j, h][:, q * 128:(q + 1) * 128],
                                    wo_h[h][:, msl],
                                    start=(h == 0), stop=(h == HL - 1))
                            ob = out_pool.tile([128, 512], f16, name="ob")
                            if last and m % 2 == 0:
                                nc.scalar.copy(ob[:], wp[:])
                            else:
                                nc.vector.tensor_copy(ob[:], wp[:])
                            eng = out_rings[ring_ctr[0] % 2]
                            ring_ctr[0] += 1
                            eng.dma_start(
                                out_d[t0 + q * 128:t0 + (q + 1) * 128, msl],
                                ob[:])

                for j, (t0, tw) in enumerate(TILES):
                    tsl = slice(t0, t0 + tw)
                    nch = min(NC_S, (t0 + tw - 1 + pcl) // 128 + 1)
                    # chunks [0, n8) sit fully below the causal boundary:
                    # exp goes straight to fp8 pair tiles (no mask needed);
                    # dn runs as fp8 DoubleRow on pairs.  Chunks [n8, nch)
                    # cross the diagonal: f16 exp + affine_select mask.
                    n8 = min(nch, max(0, (t0 + pcl - 127) // 128 + 1))
                    for h in range(HL):
                        dn = dn_ps.tile([128, 512], f32, name="dn")
                        ov = ov_ps.tile([128, 512], f32, name="ov")
                        pqs = []
                        pTs = {}
                        dn_started = [False]

                        def dn_mm(out_ap, lhs, rhs, c_last, pm=None):
                            nc.tensor.matmul(out_ap, lhs, rhs,
                                             start=not dn_started[0],
                                             stop=(c_last == nch - 1),
                                             perf_mode=pm)
                            dn_started[0] = True

                        # chunk pipeline: logits/exp at cc, denom/PV at cc-2
                        for cc in range(nch + 2):
                            if cc < nch:
                                c = cc
                                csl = slice(c * 128, (c + 1) * 128)
                                lg = lg_ps.tile([128, 512], f32, name="lg")
                                nc.tensor.matmul(
                                    lg[:, :tw], kn_pack[:, 2 * h:2 * h + 2, csl]
                                    if h == 0 else kn_pack[:, 1:3, csl],
                                    qpk_sb[:, h, :, tsl],
                                    start=True, stop=True, perf_mode=DR_MODE)
                                if c < n8:
                                    if c % 2 == 0:
                                        pqs.append(p8_pool.tile(
                                            [128, 2, 512], f8, name="pq"))
                                    nc.scalar.activation(
                                        pqs[c // 2][:, c % 2, :tw], lg[:, :tw],
                                        mybir.ActivationFunctionType.Exp,
                                        bias=0.0, scale=EXP_SCALE)
                                else:
                                    pT = p_pool.tile([128, 512], f16,
                                                     name="pT")
                                    nc.scalar.activation(
                                        pT[:, :tw], lg[:, :tw],
                                        mybir.ActivationFunctionType.Exp,
                                        bias=0.0, scale=EXP_SCALE)
                                    # crossing chunk: zero where s > t+pcl
                                    nc.gpsimd.affine_select(
                                        out=pT[:, :tw], in_=pT[:, :tw],
                                        pattern=[[1, tw]],
                                        compare_op=mybir.AluOpType.is_ge,
                                        fill=0.0,
                                        base=t0 + pcl - 128 * c,
                                        channel_multiplier=-1)
                                    pTs[c] = pT
                            if cc >= 2:
                                c = cc - 2
                                csl = slice(c * 128, (c + 1) * 128)
                                if c < n8:
                                    p_rhs = pqs[c // 2][:, c % 2, :tw]
                                    if c % 2 == 1:
                                        dn_mm(dn[:, :tw], ones8_sb[:],
                                              pqs[c // 2][:, :, :tw], c,
                                              pm=DR_MODE)
                                    elif c == n8 - 1:
                                        dn_mm(dn[:, :tw], ones8_sb[:, 0, :],
                                              p_rhs, c)
                                else:
                                    p_rhs = pTs[c][:, :tw]
                                    dn_mm(dn[:, :tw], ones_sb[:], p_rhs, c)
                                nc.tensor.matmul(ov[:, :tw], v_sb[h][:, csl],
                                                 p_rhs,
                                                 start=(c == 0),
                                                 stop=(c == nch - 1))
                        recip = r_pool.tile([128, 512], f32, name="recip")
                        nc.vector.reciprocal_approx_fast(recip[:, :tw],
                                                         dn[:, :tw])
                        o_ = ovn_pool.tile([128, 512], f16, tag="ovn",
                                           name="ovn")
                        nc.vector.tensor_mul(o_[:, :tw], ov[:, :tw],
                                             recip[:, :tw])
                        ovn_tiles[j, h] = o_
                        if h == 0 and j > 0:
                            emit_wo(j - 1)
                emit_wo(len(TILES) - 1, last=True)
    nc.compile()
    return nc


def _get_nc(pcl: int):
    if pcl not in _CACHE:
        _CACHE[pcl] = _build(pcl)
    return _CACHE[pcl]


def _prep_in_maps(q_nope, q_pe, kv_all, pe_all, wkv_b, wo):
    q_nope = np.asarray(q_nope, np.float32)
    q_pe = np.asarray(q_pe, np.float32)
    kv_all = np.asarray(kv_all, np.float32)
    pe_all = np.asarray(pe_all, np.float32)
    wkv_b = np.asarray(wkv_b, np.float32)
    wo = np.asarray(wo, np.float32)

    # latent-major coalesced layouts, block-contiguous per partition line
    kvT = kv_all.T.reshape(NCC, 128, S).transpose(1, 0, 2)   # [128, NCC, S]
    kv_blocks = np.ascontiguousarray(                        # [128,NB,NCC,512]
        kvT.reshape(128, NCC, NB, 512).transpose(0, 2, 1, 3))
    kv16 = kv_blocks.astype(np.float16)
    kv8 = (kv_blocks * S_KV).astype(np8)

    wk8 = np.ascontiguousarray(                              # [128,H,NCC,DN]
        (wkv_b[:, :DN, :] * S_WK).transpose(0, 2, 1)
        .reshape(H, NCC, 128, DN).transpose(2, 0, 1, 3)).astype(np8)
    wv16 = np.ascontiguousarray(                             # [128,H,NCC,DV]
        wkv_b[:, -DV:, :].transpose(0, 2, 1).astype(np.float16)
        .reshape(H, NCC, 128, DV).transpose(2, 0, 1, 3))

    qnT = q_nope.transpose(2, 1, 0) * S_QN                   # [128, H, T]
    qpT = np.zeros((128, H, T), np.float32)
    qpT[:DR] = q_pe.transpose(2, 1, 0) * S_QP
    qpk = np.empty((128, H, 2, T), np.float32)
    for h in range(H):
        if (h % HL) == 0:
            qpk[:, h, 0], qpk[:, h, 1] = qnT[:, h], qpT[:, h]
        else:
            qpk[:, h, 0], qpk[:, h, 1] = qpT[:, h], qnT[:, h]
    qpk8 = qpk.astype(np8)

    pe8 = np.zeros((128, S), np.float32)
    pe8[:DR] = pe_all.T * S_PE
    pe8 = pe8.astype(np8)

    ones = np.ones((128, 128), np.float16)
    ones8 = np.ones((128, 2, 128)).astype(np8)

    in_maps = []
    for core in range(NCORES):
        hs = slice(HL * core, HL * (core + 1))
        woT = np.ascontiguousarray(                          # [128, HL, DIM]
            wo[:, HL * DV * core:HL * DV * (core + 1)].T.astype(np.float16)
            .reshape(HL, 128, DIM).transpose(1, 0, 2))
        in_maps.append(dict(kv8=kv8, kv16=kv16, wk8=wk8[:, hs],
                            wv16=wv16[:, hs], qpk8=qpk8[:, hs], pe8=pe8,
                            woT=woT, ones=ones, ones8=ones8))
    return in_maps


def run(inputs: dict, trace: bool = False):
    """Run on 8 cores; returns (full_output, BassKernelResults)."""
    pcl = int(inputs["prompt_cache_len"])
    nc = _get_nc(pcl)
    in_maps = _prep_in_maps(inputs["q_nope"], inputs["q_pe"], inputs["kv_all"],
                            inputs["pe_all"], inputs["wkv_b"], inputs["wo"])
    kw = {}
    if trace:
        kw = dict(trace=True, trace_cores=list(range(NCORES)))
    res = run_bass_kernel_spmd(nc, in_maps, list(range(NCORES)), **kw)
    parts = np.stack([res.results[c]["out"] for c in range(NCORES)], 0)
    return parts.astype(np.float32).sum(0, dtype=np.float32), res


def kernel(q_nope, q_pe, kv_all, pe_all, wkv_b, wo, prompt_cache_len):
    out, _ = run(dict(q_nope=q_nope, q_pe=q_pe, kv_all=kv_all, pe_all=pe_all,
                      wkv_b=wkv_b, wo=wo, prompt_cache_len=prompt_cache_len))
    return out
